# revision 19
# baseline (speedup 1.0000x reference)
"""Multi-head attention TRN2 Bass kernel for nn_MultiHeadAttention_77610059039245.

Problem: B=4, S=2048, E=1024, H=16 heads, d_head=64, causal mask,
scale = 1/sqrt(1024). f32 inputs/outputs.

Sharding (8 cores): core c = (b, g) with b = c//2 batch, g = c%2 head-group.
Each core computes heads 8g..8g+7 of batch b end-to-end (Wq/Wk/Wv column
split, Wo row split) and returns a partial output [S, E]; the host sums the
two partials per batch (the "all-reduce").

All matmul operands are bf16 (host pre-converts x and W): enables FWL fast
weight loads, 1 cyc/row at any moving width, halves DMA + SBUF. PSUM
accumulation stays f32.

Per-core pipeline, chunked by 512 sequence columns (sc = strip it):
  for sc in 0..3:
    load x slabs (q,k,v chunk sc), project into qT/kT[pair] ([128,S] bf16,
    two heads stacked on partitions) and v_aug ([j, head, jt, 65] bf16 with a
    ones column so the softmax denominator falls out of the PV matmul);
    for each head-pair p: attention strip (p, it=sc): per j-tile jt<=4it+3,
    scores sT = kT.T @ qT (two row-tiled K=64 matmuls), exp on ACT
    (scale=1/32 folded), causal tril multiply on the diagonal tiles only,
    PV accumulate into PSUM [65, 512]; normalize via reciprocal +
    partition_broadcast + multiply into xT_out (bf16);
    output projection for s-tiles 4it..4it+3 (lhsT = xT_out, rhs = Wo part),
    staged PSUM->SBUF and DMA'd out. Scores are emitted two j-tiles ahead of
    PV so PE never waits on ACT.
"""
import numpy as np
import ml_dtypes

import concourse.bass as bass
import concourse.mybir as mybir
import concourse.tile as tile
from concourse import bacc
from concourse.bass_utils import run_bass_kernel_spmd

F32 = mybir.dt.float32
BF16 = mybir.dt.bfloat16
EXP = mybir.ActivationFunctionType.Exp

B, S, E, H = 4, 2048, 1024, 16
D = 64                    # head dim
HC = 8                    # heads per core
HP = HC // 2              # head pairs per core
GD = HC * D               # per-core projected width (512)
SCALE = 1.0 / 32.0        # 1/sqrt(QK=1024)
N_CORES = 8
CH = 512                  # s-chunk == strip width
ST = S // 128             # 16 s-tiles
IT = S // CH              # 4 strips


def build_core_kernel(reps=1):
    nc = bacc.Bacc("TRN2", target_bir_lowering=False)

    xq = nc.dram_tensor("xqT", [E, S], BF16, kind="ExternalInput")
    xk = nc.dram_tensor("xkT", [E, S], BF16, kind="ExternalInput")
    xv = nc.dram_tensor("xvT", [E, S], BF16, kind="ExternalInput")
    wq = nc.dram_tensor("wq", [E, GD], BF16, kind="ExternalInput")
    wk = nc.dram_tensor("wk", [E, GD], BF16, kind="ExternalInput")
    wv = nc.dram_tensor("wv", [E, GD], BF16, kind="ExternalInput")
    wo = nc.dram_tensor("wo", [GD, E], BF16, kind="ExternalInput")
    tril = nc.dram_tensor("tril", [128, 128], BF16, kind="ExternalInput")
    onesd = nc.dram_tensor("onesc", [128, 128], BF16, kind="ExternalInput")
    out = nc.dram_tensor("out", [S, E], BF16, kind="ExternalOutput")

    with tile.TileContext(nc) as tc:
        with (
            tc.tile_pool(name="consts", bufs=1) as consts,
            tc.tile_pool(name="wpool", bufs=32) as wpool,
            tc.tile_pool(name="slab", bufs=6) as slabp,
            tc.tile_pool(name="qkv", bufs=1) as qkv,
            tc.tile_pool(name="pt", bufs=3) as ptp,
            tc.tile_pool(name="small", bufs=2) as small,
            tc.tile_pool(name="xto", bufs=1) as xtop,
            tc.tile_pool(name="ostage", bufs=3) as ostage,
            tc.tile_pool(name="ps", bufs=3, space="PSUM") as ps,
            tc.tile_pool(name="psx", bufs=2, space="PSUM") as psx,
        ):
            # consts are DMA'd later (after the first weight stream) — they
            # aren't needed until the first strip, ~25us in
            tril_t = consts.tile([128, 128], BF16)
            ones_t = consts.tile([128, 128], BF16)
            consts_loaded = [False]

            for _rep in range(reps):
                qT = [qkv.tile([128, S], BF16, tag=f"qT{p}", name=f"qT{p}") for p in range(HP)]
                kT = [qkv.tile([128, S], BF16, tag=f"kT{p}", name=f"kT{p}") for p in range(HP)]
                # v augmented with ones column: [128, head, jt, 65]
                v_aug = qkv.tile([128, HC, ST, D + 1], BF16, tag="v_aug")
                xT_out = [xtop.tile([128, S], BF16, tag=f"xto{p}", name=f"xto{p}") for p in range(HP)]

                # ---- weights: preload everything (bf16: 32KB/partition) ----
                def load_w(wdram, ets=range(8), tiles=None):
                    tiles = list(tiles) if tiles is not None else [None] * 8
                    for et in ets:
                        t = wpool.tile([128, GD], BF16, tag="w")
                        eng = nc.sync if et % 2 == 0 else nc.scalar
                        eng.dma_start(out=t, in_=wdram[et * 128:(et + 1) * 128, :])
                        tiles[et] = t
                    return tiles

                def transpose_chunk(xdram, sc, eng2=None):
                    """DMA xT columns [sc*CH, (sc+1)*CH) into a slab [128, 8, CH].

                    eng2 defaults to sync: prefetches issued mid-strip must not
                    ride the ACT queue (it carries the exp stream)."""
                    slab = slabp.tile([128, 8, CH], BF16, tag="slab")
                    src = xdram.rearrange("(a p) s -> p a s", p=128)
                    (eng2 or nc.sync).dma_start(
                        out=slab[:, 0:4], in_=src[:, 0:4, sc * CH:(sc + 1) * CH])
                    nc.sync.dma_start(
                        out=slab[:, 4:8], in_=src[:, 4:8, sc * CH:(sc + 1) * CH])
                    return slab

                def proj_qk(dest, wt, slab, sc):
                    for dp in range(HP):
                        pj = ps.tile([128, 2, 512], F32, tag="ps", name="pj")
                        for et in range(8):
                            nc.tensor.matmul(
                                pj[:, 0, :],
                                wt[et][:, dp * 128:(dp + 1) * 128],
                                slab[:, et, :],
                                start=(et == 0), stop=(et == 7))
                        nc.vector.tensor_copy(
                            dest[dp][:, sc * CH:(sc + 1) * CH], pj[:, 0, :])

                def proj_v(wt, slab, sc):
                    for st in range(CH // 128):
                        jt = sc * (CH // 128) + st
                        pj = ps.tile([128, 2, 512], F32, tag="ps", name="pj")
                        for et in range(8):
                            nc.tensor.matmul(
                                pj[:, 0, :],
                                slab[:, et, st * 128:(st + 1) * 128],
                                wt[et],
                                start=(et == 0), stop=(et == 7))
                        nc.vector.tensor_copy(
                            v_aug[:, :, jt, 0:D],
                            pj[:, 0, :].rearrange("p (h d) -> p h d", h=HC))

                # ---- attention strip helpers ----
                def emit_scores(p, it, jt):
                    kdiag = jt - 4 * it
                    c0 = 128 * kdiag if kdiag > 0 else 0
                    i0 = it * 512 + c0
                    sw = ps.tile([128, 2, 512], F32, tag="ps", name="sw")
                    nc.tensor.matmul(
                        sw[:, 0, c0:], kT[p][0:64, jt * 128:(jt + 1) * 128],
                        qT[p][0:64, i0:(it + 1) * 512],
                        start=True, stop=True)
                    nc.tensor.matmul(
                        sw[:, 1, c0:], kT[p][64:128, jt * 128:(jt + 1) * 128],
                        qT[p][64:128, i0:(it + 1) * 512],
                        start=True, stop=True)
                    return sw, c0

                def emit_normalize(p, it, px1, px2, act_recip=False):
                    # act_recip: at chunk boundaries ACT is idle (no exp) and
                    # outproj waits on this chain — use ACT for the recips so
                    # both heads' chains overlap with the DVE muls.
                    rrows = []
                    for hh, px in ((0, px1), (1, px2)):
                        rrow = small.tile([1, 512], F32, tag="rrow", name="rrow")
                        nc.vector.reciprocal(rrow, px[64:65, :])
                        rrows.append(rrow)
                    for hh, px in ((0, px1), (1, px2)):
                        bc = small.tile([64, 512], F32, tag="bc", name="bc")
                        nc.gpsimd.partition_broadcast(bc, rrows[hh])
                        nc.vector.tensor_mul(
                            xT_out[p][hh * 64:(hh + 1) * 64,
                                      it * 512:(it + 1) * 512],
                            px[0:64, :], bc)

                state = {"pending": None}   # (p, it, px1, px2) awaiting normalize

                def emit_strip(p, it):
                    """Scores two j-tiles ahead of PV so PE never waits on ACT.
                    The previous strip's normalize is emitted after this strip's
                    first scores so DVE gets a head start on the psx WAR."""
                    h1, h2 = 2 * p, 2 * p + 1
                    jmax = 4 * it + 3
                    px1 = psx.tile([128, 512], F32, tag="psx", name="px1")
                    px2 = psx.tile([128, 512], F32, tag="psx", name="px2")
                    pend = []          # [(sw, c0), ...] scores not yet consumed
                    pend.append(emit_scores(p, it, 0))
                    if state["pending"] is not None:
                        emit_normalize(*state["pending"])
                        state["pending"] = None
                    if jmax >= 1:
                        pend.append(emit_scores(p, it, 1))
                    for jt in range(jmax + 1):
                        sw_cur, c0 = pend.pop(0)
                        pt = ptp.tile([128, 2, 512], BF16, tag="pt", name="pt")
                        nc.scalar.activation(pt[:, :, c0:], sw_cur[:, :, c0:],
                                             EXP, scale=SCALE)
                        kdiag = jt - 4 * it
                        if kdiag >= 0:
                            cs = slice(c0, c0 + 128)
                            nc.vector.tensor_mul(pt[:, 0, cs], pt[:, 0, cs], tril_t)
                            nc.vector.tensor_mul(pt[:, 1, cs], pt[:, 1, cs], tril_t)
                        if jt + 2 <= jmax:
                            pend.append(emit_scores(p, it, jt + 2))
                        nc.tensor.matmul(
                            px1[0:65, c0:], v_aug[:, h1, jt, :], pt[:, 0, c0:],
                            start=(jt == 0), stop=(jt == jmax))
                        nc.tensor.matmul(
                            px2[0:65, c0:], v_aug[:, h2, jt, :], pt[:, 1, c0:],
                            start=(jt == 0), stop=(jt == jmax))
                    state["pending"] = (p, it, px1, px2)

                def flush_normalize():
                    if state["pending"] is not None:
                        emit_normalize(*state["pending"], act_recip=True)
                        state["pending"] = None

                def emit_outproj_block(it, wot):
                    # pending normalize feeds this block's kt=3 matmuls; emit it
                    # first (outproj's DVE copy would otherwise queue ahead of
                    # it on DVE and deadlock). The first two s-tiles emit their
                    # kt=0..2 matmuls before either kt=3 so PE covers the
                    # normalize latency.
                    flush_normalize()
                    sts = list(range(4 * it, 4 * it + 4))

                    def po_mms(po, st, kts):
                        for eh in range(2):
                            for kt in kts:
                                nc.tensor.matmul(
                                    po[:, eh, :],
                                    xT_out[kt][:, st * 128:(st + 1) * 128],
                                    wot[kt * 2 + eh],
                                    start=(kt == 0), stop=(kt == 3))

                    def po_store(po, st):
                        # GPSIMD has no PSUM port — staging stays on DVE
                        ot = ostage.tile([128, 1024], BF16, tag="ostage")
                        nc.vector.tensor_copy(ot, po.rearrange("p a b -> p (a b)"))
                        eng = nc.sync if st % 2 == 0 else nc.scalar
                        eng.dma_start(out=out[st * 128:(st + 1) * 128, :], in_=ot)

                    po0 = ps.tile([128, 2, 512], F32, tag="ps", name="po")
                    po_mms(po0, sts[0], (0, 1, 2))
                    po1 = ps.tile([128, 2, 512], F32, tag="ps", name="po")
                    po_mms(po1, sts[1], (0, 1, 2))
                    po_mms(po0, sts[0], (3,))
                    po_store(po0, sts[0])
                    po_mms(po1, sts[1], (3,))
                    po_store(po1, sts[1])
                    for st in sts[2:]:
                        po = ps.tile([128, 2, 512], F32, tag="ps", name="po")
                        po_mms(po, st, (0, 1, 2, 3))
                        po_store(po, st)

                # ---- fused schedule ----
                with nc.named_scope("mha"):
                    # startup: first two wq tiles, then the first slab, then
                    # the rest — the et=0 matmul can start as soon as its
                    # slab half + wq[0] land
                    wts = {"q": load_w(wq, ets=(0, 1))}
                    slab_q0 = transpose_chunk(xq, 0, eng2=nc.scalar)
                    wts["q"] = load_w(wq, ets=range(2, 8), tiles=wts["q"])
                    slab_k0 = transpose_chunk(xk, 0, eng2=nc.scalar)
                    wts["k"] = load_w(wk)
                    slab_v0 = transpose_chunk(xv, 0, eng2=nc.scalar)
                    wts["v"] = load_w(wv)
                    if not consts_loaded[0]:
                        nc.sync.dma_start(out=tril_t, in_=tril[:, :])
                        nc.scalar.dma_start(out=ones_t, in_=onesd[:, :])
                        consts_loaded[0] = True
                    nc.gpsimd.tensor_copy(
                        v_aug[:, :, :, D:D + 1],
                        ones_t.rearrange("p (a b c) -> p a b c", a=HC, b=ST))
                    wot = []
                    for kt in range(4):
                        for eh in range(2):
                            t = wpool.tile([128, GD], BF16, tag="w")
                            eng = nc.sync if eh == 0 else nc.scalar
                            eng.dma_start(
                                out=t, in_=wo[kt * 128:(kt + 1) * 128,
                                              eh * 512:(eh + 1) * 512])
                            wot.append(t)

                    slabs_next = [slab_q0, slab_k0, slab_v0]
                    for sc in range(IT):
                        slabs_cur = slabs_next
                        if sc + 1 < IT:
                            slabs_next = [transpose_chunk(xq, sc + 1),
                                          transpose_chunk(xk, sc + 1),
                                          transpose_chunk(xv, sc + 1)]
                        proj_qk(qT, wts["q"], slabs_cur[0], sc)
                        proj_qk(kT, wts["k"], slabs_cur[1], sc)
                        proj_v(wts["v"], slabs_cur[2], sc)
                        for p in range(HP):
                            emit_strip(p, sc)
                        emit_outproj_block(sc, wot)

    nc.finalize()
    return nc


_NC = None


def _get_nc():
    global _NC
    if _NC is None:
        _NC = build_core_kernel()
    return _NC


def _tril_mask():
    # multiplicative causal mask for s^T blocks: keep j (row) <= i (col)
    r = np.arange(128)
    return np.where(r[:, None] <= r[None, :], 1.0, 0.0).astype(ml_dtypes.bfloat16)


def make_in_maps(query, key, value, Wq, Wk, Wv, Wo):
    bf = ml_dtypes.bfloat16
    query = np.asarray(query, np.float32)
    key = np.asarray(key, np.float32)
    value = np.asarray(value, np.float32)
    Wq = np.asarray(Wq, np.float32).astype(bf)
    Wk = np.asarray(Wk, np.float32).astype(bf)
    Wv = np.asarray(Wv, np.float32).astype(bf)
    Wo = np.asarray(Wo, np.float32).astype(bf)
    tril_m = _tril_mask()
    ones_m = np.ones((128, 128), bf)
    xqT = [np.ascontiguousarray(query[b].T.astype(bf)) for b in range(B)]
    xkT = [np.ascontiguousarray(key[b].T.astype(bf)) for b in range(B)]
    xvT = [np.ascontiguousarray(value[b].T.astype(bf)) for b in range(B)]
    in_maps = []
    for c in range(N_CORES):
        b, g = c // 2, c % 2
        cols = slice(g * GD, (g + 1) * GD)
        in_maps.append({
            "xqT": xqT[b],
            "xkT": xkT[b],
            "xvT": xvT[b],
            "wq": np.ascontiguousarray(Wq[:, cols]),
            "wk": np.ascontiguousarray(Wk[:, cols]),
            "wv": np.ascontiguousarray(Wv[:, cols]),
            "wo": np.ascontiguousarray(Wo[g * GD:(g + 1) * GD, :]),
            "tril": tril_m,
            "onesc": ones_m,
        })
    return in_maps


def kernel(query, key, value, mask, Wq, Wk, Wv, Wo, **run_kwargs):
    nc = _get_nc()
    in_maps = make_in_maps(query, key, value, Wq, Wk, Wv, Wo)
    res = run_bass_kernel_spmd(nc, in_maps, core_ids=list(range(N_CORES)),
                               **run_kwargs)
    out = np.empty((B, S, E), np.float32)
    for b in range(B):
        out[b] = (res.results[2 * b]["out"].astype(np.float32)
                  + res.results[2 * b + 1]["out"].astype(np.float32))
    if run_kwargs:
        kernel.last_result = res
    return out


if __name__ == "__main__":
    rng = np.random.default_rng(0)
    q = rng.standard_normal((B, S, E), dtype=np.float32)
    k = rng.standard_normal((B, S, E), dtype=np.float32)
    v = rng.standard_normal((B, S, E), dtype=np.float32)
    sc = 1.0 / np.sqrt(E)
    Wq = rng.standard_normal((E, E), dtype=np.float32) * sc
    Wk = rng.standard_normal((E, E), dtype=np.float32) * sc
    Wv = rng.standard_normal((E, E), dtype=np.float32) * sc
    Wo = rng.standard_normal((E, E), dtype=np.float32) * sc
    o = kernel(q, k, v, None, Wq, Wk, Wv, Wo)
    print("out", o.shape, o.dtype, float(np.abs(o).mean()))


# revision 39
# speedup vs baseline: 1.1911x; 1.1911x over previous
"""Multi-head attention TRN2 Bass kernel for nn_MultiHeadAttention_77610059039245.

Problem: B=4, S=2048, E=1024, H=16 heads, d_head=64, causal mask,
scale = 1/sqrt(1024). f32 inputs/outputs.

Sharding (8 cores): core c = (b, g) with b = c//2 batch, g = c%2 head-group.
Each core computes heads 8g..8g+7 of batch b end-to-end (Wq/Wk/Wv column
split, Wo row split) and returns a partial output [S, E]; the host sums the
two partials per batch (the "all-reduce").

All matmul operands are bf16 (host pre-converts x and W): enables FWL fast
weight loads, 1 cyc/row at any moving width, halves DMA + SBUF. PSUM
accumulation stays f32.

Per-core pipeline, chunked by 512 sequence columns (sc = strip it):
  for sc in 0..3:
    load x slabs (q,k,v chunk sc), project into qT/kT[pair] ([128,S] bf16,
    two heads stacked on partitions) and v_aug ([j, head, jt, 65] bf16 with a
    ones column so the softmax denominator falls out of the PV matmul);
    for each head-pair p: attention strip (p, it=sc): per j-tile jt<=4it+3,
    scores sT = kT.T @ qT (two row-tiled K=64 matmuls), exp on ACT
    (scale=1/32 folded), causal tril multiply on the diagonal tiles only,
    PV accumulate into PSUM [65, 512]; normalize via reciprocal +
    partition_broadcast + multiply into xT_out (bf16);
    output projection for s-tiles 4it..4it+3 (lhsT = xT_out, rhs = Wo part),
    staged PSUM->SBUF and DMA'd out. Scores are emitted two j-tiles ahead of
    PV so PE never waits on ACT.
"""
import numpy as np
import ml_dtypes

import concourse.bass as bass
import concourse.mybir as mybir
import concourse.tile as tile
from concourse import bacc
from concourse.bass_utils import run_bass_kernel_spmd

F32 = mybir.dt.float32
BF16 = mybir.dt.bfloat16
EXP = mybir.ActivationFunctionType.Exp

B, S, E, H = 4, 2048, 1024, 16
D = 64                    # head dim
HC = 8                    # heads per core
HP = HC // 2              # head pairs per core
GD = HC * D               # per-core projected width (512)
SCALE = 1.0 / 32.0        # 1/sqrt(QK=1024)
N_CORES = 8
CH = 512                  # s-chunk == strip width
ST = S // 128             # 16 s-tiles
IT = S // CH              # 4 strips


def build_core_kernel(reps=1):
    nc = bacc.Bacc("TRN2", target_bir_lowering=False)

    xq = nc.dram_tensor("xqT", [E, S], BF16, kind="ExternalInput")
    xk = nc.dram_tensor("xkT", [E, S], BF16, kind="ExternalInput")
    xv = nc.dram_tensor("xvT", [E, S], BF16, kind="ExternalInput")
    wq = nc.dram_tensor("wq", [E, GD], BF16, kind="ExternalInput")
    wk = nc.dram_tensor("wk", [E, GD], BF16, kind="ExternalInput")
    wv = nc.dram_tensor("wv", [E, GD], BF16, kind="ExternalInput")
    wo = nc.dram_tensor("wo", [GD, E], BF16, kind="ExternalInput")
    identd = nc.dram_tensor("identd", [128, 128], BF16, kind="ExternalInput")
    masknd = nc.dram_tensor("maskneg", [128, 128], BF16, kind="ExternalInput")
    onesd = nc.dram_tensor("onesc", [128, 128], BF16, kind="ExternalInput")
    out = nc.dram_tensor("out", [S, E], BF16, kind="ExternalOutput")

    with tile.TileContext(nc) as tc:
        with (
            tc.tile_pool(name="consts", bufs=1) as consts,
            tc.tile_pool(name="wpool", bufs=32) as wpool,
            tc.tile_pool(name="slab", bufs=6) as slabp,
            tc.tile_pool(name="qkv", bufs=1) as qkv,
            tc.tile_pool(name="pt", bufs=3) as ptp,
            tc.tile_pool(name="small", bufs=2) as small,
            tc.tile_pool(name="xto", bufs=1) as xtop,
            tc.tile_pool(name="ostage", bufs=3) as ostage,
            tc.tile_pool(name="ps", bufs=3, space="PSUM") as ps,
            tc.tile_pool(name="psx", bufs=2, space="PSUM") as psx,
        ):
            # consts are DMA'd later (after the first weight stream) — they
            # aren't needed until the first strip, ~25us in
            ident_t = consts.tile([128, 128], BF16, tag="ident")
            maskn_t = consts.tile([128, 128], BF16, tag="maskn")
            ones_t = consts.tile([128, 128], BF16, tag="ones")
            consts_loaded = [False]

            for _rep in range(reps):
                qT = [qkv.tile([128, S], BF16, tag=f"qT{p}", name=f"qT{p}") for p in range(HP)]
                kT = [qkv.tile([128, S], BF16, tag=f"kT{p}", name=f"kT{p}") for p in range(HP)]
                # v augmented with ones column: [128, head, jt, 65]
                # per-chunk tiles: dep tracking is tile-granular, so later
                # proj writes must not alias tiles earlier stages read
                v_aug = [qkv.tile([128, HC, CH // 128, D + 1], BF16,
                                  tag=f"v_aug{sc}", name=f"v_aug{sc}")
                         for sc in range(IT)]
                xT_out = [[xtop.tile([128, CH], BF16, tag=f"xto{p}_{sc}",
                                     name=f"xto{p}_{sc}")
                           for sc in range(IT)] for p in range(HP)]

                # ---- weights: preload everything (bf16: 32KB/partition) ----
                def load_w(wdram, ets=range(8), tiles=None):
                    tiles = list(tiles) if tiles is not None else [None] * 8
                    for et in ets:
                        t = wpool.tile([128, GD], BF16, tag="w")
                        eng = nc.sync if et % 2 == 0 else nc.scalar
                        eng.dma_start(out=t, in_=wdram[et * 128:(et + 1) * 128, :])
                        tiles[et] = t
                    return tiles

                def transpose_chunk(xdram, sc, eng2=None):
                    """DMA xT columns [sc*CH, (sc+1)*CH) into a slab [128, 8, CH].

                    eng2 defaults to sync: prefetches issued mid-strip must not
                    ride the ACT queue (it carries the exp stream)."""
                    slab = slabp.tile([128, 8, CH], BF16, tag="slab")
                    src = xdram.rearrange("(a p) s -> p a s", p=128)
                    (eng2 or nc.sync).dma_start(
                        out=slab[:, 0:4], in_=src[:, 0:4, sc * CH:(sc + 1) * CH])
                    nc.sync.dma_start(
                        out=slab[:, 4:8], in_=src[:, 4:8, sc * CH:(sc + 1) * CH])
                    return slab

                def proj_qk(dest, wt, slab, sc):
                    for dp in range(HP):
                        pj = ps.tile([128, 2, 512], F32, tag="ps", name="pj")
                        for et in range(8):
                            nc.tensor.matmul(
                                pj[:, 0, :],
                                wt[et][:, dp * 128:(dp + 1) * 128],
                                slab[:, et, :],
                                start=(et == 0), stop=(et == 7))
                        nc.vector.tensor_copy(
                            dest[dp][:, sc * CH:(sc + 1) * CH], pj[:, 0, :])

                def proj_v(wt, slab, sc):
                    for st in range(CH // 128):
                        pj = ps.tile([128, 2, 512], F32, tag="ps", name="pj")
                        for et in range(8):
                            nc.tensor.matmul(
                                pj[:, 0, :],
                                slab[:, et, st * 128:(st + 1) * 128],
                                wt[et],
                                start=(et == 0), stop=(et == 7))
                        nc.vector.tensor_copy(
                            v_aug[sc][:, :, st, 0:D],
                            pj[:, 0, :].rearrange("p (h d) -> p h d", h=HC))

                # ---- attention strip helpers ----
                def emit_scores(p, it, jt):
                    kdiag = jt - 4 * it
                    c0 = 128 * kdiag if kdiag > 0 else 0
                    i0 = it * 512 + c0
                    diag = kdiag >= 0
                    sw = ps.tile([128, 2, 512], F32, tag="ps", name="sw")
                    for hh in range(2):
                        nc.tensor.matmul(
                            sw[:, hh, c0:],
                            kT[p][hh * 64:(hh + 1) * 64, jt * 128:(jt + 1) * 128],
                            qT[p][hh * 64:(hh + 1) * 64, i0:(it + 1) * 512],
                            start=True, stop=not diag,
                            skip_group_check=diag)
                    if diag:
                        # additive causal mask on the diagonal block: the
                        # matmul adds maskneg[j, i] (-1e5 where j > i) into the
                        # first 128 score columns; exp then zeroes them
                        for hh in range(2):
                            nc.tensor.matmul(
                                sw[:, hh, c0:c0 + 128], ident_t, maskn_t,
                                start=False, stop=True, skip_group_check=True)
                    return sw, c0

                def emit_normalize(p, it, px1, px2, act_recip=False):
                    # act_recip: at chunk boundaries ACT is idle (no exp) and
                    # outproj waits on this chain — use ACT for the recips so
                    # both heads' chains overlap with the DVE muls.
                    rrows = []
                    for hh, px in ((0, px1), (1, px2)):
                        rrow = small.tile([1, 512], F32, tag="rrow", name="rrow")
                        nc.vector.reciprocal(rrow, px[64:65, :])
                        rrows.append(rrow)
                    for hh, px in ((0, px1), (1, px2)):
                        bc = small.tile([64, 512], F32, tag="bc", name="bc")
                        nc.gpsimd.partition_broadcast(bc, rrows[hh])
                        nc.vector.tensor_mul(
                            xT_out[p][it][hh * 64:(hh + 1) * 64, :],
                            px[0:64, :], bc)

                def emit_pv(p, it, jt, px1, px2, pt, c0):
                    h1, h2 = 2 * p, 2 * p + 1
                    jmax = 4 * it + 3
                    nc.tensor.matmul(
                        px1[0:65, c0:], v_aug[jt // 4][:, h1, jt % 4, :],
                        pt[:, 0, c0:],
                        start=(jt == 0), stop=(jt == jmax))
                    nc.tensor.matmul(
                        px2[0:65, c0:], v_aug[jt // 4][:, h2, jt % 4, :],
                        pt[:, 1, c0:],
                        start=(jt == 0), stop=(jt == jmax))

                def emit_outproj_tile(st, wot, split_store=False):
                    po = ps.tile([128, 2, 512], F32, tag="ps", name="po")
                    blk, col = st // 4, (st % 4) * 128
                    for eh in range(2):
                        for kt in range(4):
                            nc.tensor.matmul(
                                po[:, eh, :],
                                xT_out[kt][blk][:, col:col + 128],
                                wot[kt * 2 + eh],
                                start=(kt == 0), stop=(kt == 3))
                    # GPSIMD has no PSUM port — staging stays on DVE
                    ot = ostage.tile([128, 1024], BF16, tag="ostage")
                    if split_store:
                        nc.vector.tensor_copy(ot[:, 0:512], po[:, 0, :])
                        nc.sync.dma_start(
                            out=out[st * 128:(st + 1) * 128, 0:512],
                            in_=ot[:, 0:512])
                        nc.vector.tensor_copy(ot[:, 512:1024], po[:, 1, :])
                        nc.scalar.dma_start(
                            out=out[st * 128:(st + 1) * 128, 512:1024],
                            in_=ot[:, 512:1024])
                    else:
                        nc.vector.tensor_copy(ot, po.rearrange("p a b -> p (a b)"))
                        eng = nc.sync if st % 2 == 0 else nc.scalar
                        eng.dma_start(out=out[st * 128:(st + 1) * 128, :], in_=ot)

                def emit_qproj_group(sc, dp, slab):
                    pj = ps.tile([128, 2, 512], F32, tag="ps", name="pj")
                    for et in range(8):
                        nc.tensor.matmul(
                            pj[:, 0, :],
                            wts["q"][et][:, dp * 128:(dp + 1) * 128],
                            slab[:, et, :],
                            start=(et == 0), stop=(et == 7))
                    nc.vector.tensor_copy(
                        qT[dp][:, sc * CH:(sc + 1) * CH], pj[:, 0, :])

                # ---- fused schedule ----
                with nc.named_scope("mha"):
                    # startup: first two wq tiles, then the first slab, then
                    # the rest — the et=0 matmul can start as soon as its
                    # slab half + wq[0] land
                    wts = {"q": load_w(wq, ets=(0, 1))}
                    slab_q0 = transpose_chunk(xq, 0, eng2=nc.scalar)
                    wts["q"] = load_w(wq, ets=range(2, 8), tiles=wts["q"])
                    slab_k0 = transpose_chunk(xk, 0, eng2=nc.scalar)
                    wts["k"] = load_w(wk)
                    slab_v0 = transpose_chunk(xv, 0, eng2=nc.scalar)
                    wts["v"] = load_w(wv)
                    if not consts_loaded[0]:
                        nc.sync.dma_start(out=ident_t, in_=identd[:, :])
                        nc.scalar.dma_start(out=maskn_t, in_=masknd[:, :])
                        nc.scalar.dma_start(out=ones_t, in_=onesd[:, :])
                        consts_loaded[0] = True
                    for sc in range(IT):
                        nc.gpsimd.tensor_copy(
                            v_aug[sc][:, :, :, D:D + 1],
                            ones_t[:, 0:HC * (CH // 128)].rearrange(
                                "p (a b c) -> p a b c", a=HC, b=CH // 128))
                    wot = []
                    for kt in range(4):
                        for eh in range(2):
                            t = wpool.tile([128, GD], BF16, tag="w")
                            eng = nc.sync if eh == 0 else nc.scalar
                            eng.dma_start(
                                out=t, in_=wo[kt * 128:(kt + 1) * 128,
                                              eh * 512:(eh + 1) * 512])
                            wot.append(t)

                    # ---- one continuous attention pipeline ----
                    # all (pair, chunk, j-tile) stages form a single
                    # depth-2 score->exp->PV pipeline (ACT never drains at
                    # pair/chunk boundaries). k/v projection blocks are
                    # emitted as barriers right before the first scores of
                    # their chunk; pair transitions carry normalize +
                    # outproj-tile (chunk sc-1) + q-proj (chunk sc+1)
                    # fillers that keep PE busy while ACT catches up.
                    proj_qk(qT, wts["q"], slab_q0, 0)
                    proj_qk(kT, wts["k"], slab_k0, 0)
                    proj_v(wts["v"], slab_v0, 0)
                    slabs = {0: [slab_q0, slab_k0, slab_v0]}
                    slabs[1] = [transpose_chunk(xq, 1),
                                transpose_chunk(xk, 1),
                                transpose_chunk(xv, 1)]

                    stages = [(p, sc, jt)
                              for sc in range(IT)
                              for p in range(HP)
                              for jt in range(4 * sc + 4)]
                    first_of_chunk = {}
                    for idx, (p, sc, jt) in enumerate(stages):
                        first_of_chunk.setdefault(sc, idx)

                    pend = []      # [(sw, c0), ...] scores not yet consumed
                    px_cur = {}    # pair -> (px1, px2)

                    def emit_stage_scores(idx):
                        p, sc, jt = stages[idx]
                        if sc >= 1 and idx == first_of_chunk[sc]:
                            # barrier: this chunk's k/v projections (q was
                            # filled at chunk sc-1 pair transitions)
                            proj_qk(kT, wts["k"], slabs[sc][1], sc)
                            proj_v(wts["v"], slabs[sc][2], sc)
                            if sc + 1 < IT:
                                slabs[sc + 1] = [transpose_chunk(xq, sc + 1),
                                                 transpose_chunk(xk, sc + 1),
                                                 transpose_chunk(xv, sc + 1)]
                        pend.append(emit_scores(p, sc, jt))

                    emit_stage_scores(0)
                    emit_stage_scores(1)
                    for i, (p, sc, jt) in enumerate(stages):
                        sw_cur, c0 = pend.pop(0)
                        pt = ptp.tile([128, 2, 512], BF16, tag="pt", name="pt")
                        nc.scalar.activation(pt[:, :, c0:], sw_cur[:, :, c0:],
                                             EXP, scale=SCALE)
                        if i + 2 < len(stages):
                            emit_stage_scores(i + 2)
                        if jt == 0:
                            px_cur[p] = (
                                psx.tile([128, 512], F32, tag="psx", name="px1"),
                                psx.tile([128, 512], F32, tag="psx", name="px2"))
                        px1, px2 = px_cur[p]
                        emit_pv(p, sc, jt, px1, px2, pt, c0)
                        if jt == 4 * sc + 3:
                            # pair transition: normalize, then filler blocks.
                            # outproj tiles all go to chunk-3 transitions
                            # (3 each) — that's where ACT-pacing leaves PE
                            # idle and no q-proj filler remains.
                            emit_normalize(p, sc, px1, px2)
                            if sc + 1 < IT:
                                emit_qproj_group(sc + 1, p, slabs[sc + 1][0])
                            if sc == IT - 1:
                                for st in range(3 * p, 3 * p + 3):
                                    emit_outproj_tile(st, wot)
                    for st in range(4 * (IT - 1), 4 * IT):
                        emit_outproj_tile(st, wot, split_store=True)

    nc.finalize()
    return nc


_NC = None


def _get_nc():
    global _NC
    if _NC is None:
        _NC = build_core_kernel()
    return _NC


def _maskneg():
    # additive causal mask for s^T blocks: -1e5 where j (row) > i (col)
    r = np.arange(128)
    return np.where(r[:, None] > r[None, :], -1.0e5, 0.0).astype(ml_dtypes.bfloat16)


def make_in_maps(query, key, value, Wq, Wk, Wv, Wo):
    bf = ml_dtypes.bfloat16
    query = np.asarray(query, np.float32)
    key = np.asarray(key, np.float32)
    value = np.asarray(value, np.float32)
    Wq = np.asarray(Wq, np.float32).astype(bf)
    Wk = np.asarray(Wk, np.float32).astype(bf)
    Wv = np.asarray(Wv, np.float32).astype(bf)
    Wo = np.asarray(Wo, np.float32).astype(bf)
    maskn_m = _maskneg()
    ident_m = np.eye(128, dtype=np.float32).astype(bf)
    ones_m = np.ones((128, 128), bf)
    xqT = [np.ascontiguousarray(query[b].T.astype(bf)) for b in range(B)]
    xkT = [np.ascontiguousarray(key[b].T.astype(bf)) for b in range(B)]
    xvT = [np.ascontiguousarray(value[b].T.astype(bf)) for b in range(B)]
    in_maps = []
    for c in range(N_CORES):
        b, g = c // 2, c % 2
        cols = slice(g * GD, (g + 1) * GD)
        in_maps.append({
            "xqT": xqT[b],
            "xkT": xkT[b],
            "xvT": xvT[b],
            "wq": np.ascontiguousarray(Wq[:, cols]),
            "wk": np.ascontiguousarray(Wk[:, cols]),
            "wv": np.ascontiguousarray(Wv[:, cols]),
            "wo": np.ascontiguousarray(Wo[g * GD:(g + 1) * GD, :]),
            "identd": ident_m,
            "maskneg": maskn_m,
            "onesc": ones_m,
        })
    return in_maps


def kernel(query, key, value, mask, Wq, Wk, Wv, Wo, **run_kwargs):
    nc = _get_nc()
    in_maps = make_in_maps(query, key, value, Wq, Wk, Wv, Wo)
    res = run_bass_kernel_spmd(nc, in_maps, core_ids=list(range(N_CORES)),
                               **run_kwargs)
    out = np.empty((B, S, E), np.float32)
    for b in range(B):
        out[b] = (res.results[2 * b]["out"].astype(np.float32)
                  + res.results[2 * b + 1]["out"].astype(np.float32))
    if run_kwargs:
        kernel.last_result = res
    return out


if __name__ == "__main__":
    rng = np.random.default_rng(0)
    q = rng.standard_normal((B, S, E), dtype=np.float32)
    k = rng.standard_normal((B, S, E), dtype=np.float32)
    v = rng.standard_normal((B, S, E), dtype=np.float32)
    sc = 1.0 / np.sqrt(E)
    Wq = rng.standard_normal((E, E), dtype=np.float32) * sc
    Wk = rng.standard_normal((E, E), dtype=np.float32) * sc
    Wv = rng.standard_normal((E, E), dtype=np.float32) * sc
    Wo = rng.standard_normal((E, E), dtype=np.float32) * sc
    o = kernel(q, k, v, None, Wq, Wk, Wv, Wo)
    print("out", o.shape, o.dtype, float(np.abs(o).mean()))


# revision 41
# speedup vs baseline: 2.2974x; 1.9288x over previous
"""Multi-head attention TRN2 Bass kernel for nn_MultiHeadAttention_77610059039245.

Problem: B=4, S=2048, E=1024, H=16 heads, d_head=64, causal mask,
scale = 1/sqrt(1024). f32 inputs/outputs.

Sharding (8 cores): core c = (b, g) with b = c//2 batch, g = c%2 head-group.
Each core computes heads 8g..8g+7 of batch b end-to-end (Wq/Wk/Wv column
split, Wo row split) and returns a partial output [S, E]; the host sums the
two partials per batch (the "all-reduce").

All matmul operands are bf16 (host pre-converts x and W): enables FWL fast
weight loads, 1 cyc/row at any moving width, halves DMA + SBUF. PSUM
accumulation stays f32.

Per-core pipeline, chunked by 512 sequence columns (sc = strip it):
  for sc in 0..3:
    load x slabs (q,k,v chunk sc), project into qT/kT[pair] ([128,S] bf16,
    two heads stacked on partitions) and v_aug ([j, head, jt, 65] bf16 with a
    ones column so the softmax denominator falls out of the PV matmul);
    for each head-pair p: attention strip (p, it=sc): per j-tile jt<=4it+3,
    scores sT = kT.T @ qT (two row-tiled K=64 matmuls), exp on ACT
    (scale=1/32 folded), causal tril multiply on the diagonal tiles only,
    PV accumulate into PSUM [65, 512]; normalize via reciprocal +
    partition_broadcast + multiply into xT_out (bf16);
    output projection for s-tiles 4it..4it+3 (lhsT = xT_out, rhs = Wo part),
    staged PSUM->SBUF and DMA'd out. Scores are emitted two j-tiles ahead of
    PV so PE never waits on ACT.
"""
import numpy as np
import ml_dtypes

import concourse.bass as bass
import concourse.mybir as mybir
import concourse.tile as tile
from concourse import bacc
from concourse.bass_utils import run_bass_kernel_spmd

F32 = mybir.dt.float32
BF16 = mybir.dt.bfloat16
EXP = mybir.ActivationFunctionType.Exp

B, S, E, H = 4, 2048, 1024, 16
D = 64                    # head dim
HC = 8                    # heads per core
HP = HC // 2              # head pairs per core
GD = HC * D               # per-core projected width (512)
SCALE = 1.0 / 32.0        # 1/sqrt(QK=1024)
N_CORES = 8
CH = 512                  # s-chunk == strip width
ST = S // 128             # 16 s-tiles
IT = S // CH              # 4 strips


def build_core_kernel(reps=1):
    nc = bacc.Bacc("TRN2", target_bir_lowering=False)

    xq = nc.dram_tensor("xqT", [E, S], BF16, kind="ExternalInput")
    xk = nc.dram_tensor("xkT", [E, S], BF16, kind="ExternalInput")
    xv = nc.dram_tensor("xvT", [E, S], BF16, kind="ExternalInput")
    wq = nc.dram_tensor("wq", [E, GD], BF16, kind="ExternalInput")
    wk = nc.dram_tensor("wk", [E, GD], BF16, kind="ExternalInput")
    wv = nc.dram_tensor("wv", [E, GD], BF16, kind="ExternalInput")
    wo = nc.dram_tensor("wo", [GD, E], BF16, kind="ExternalInput")
    identd = nc.dram_tensor("identd", [128, 128], BF16, kind="ExternalInput")
    masknd = nc.dram_tensor("maskneg", [128, 128], BF16, kind="ExternalInput")
    onesd = nc.dram_tensor("onesc", [128, 128], BF16, kind="ExternalInput")
    out = nc.dram_tensor("out", [S, E], BF16, kind="ExternalOutput")

    with tile.TileContext(nc) as tc:
        with (
            tc.tile_pool(name="consts", bufs=1) as consts,
            tc.tile_pool(name="wpool", bufs=32) as wpool,
            tc.tile_pool(name="slab", bufs=6) as slabp,
            tc.tile_pool(name="qkv", bufs=1) as qkv,
            tc.tile_pool(name="pt", bufs=3) as ptp,
            tc.tile_pool(name="small", bufs=2) as small,
            tc.tile_pool(name="xto", bufs=1) as xtop,
            tc.tile_pool(name="ostage", bufs=3) as ostage,
            tc.tile_pool(name="ps", bufs=3, space="PSUM") as ps,
            tc.tile_pool(name="psx", bufs=2, space="PSUM") as psx,
        ):
            # consts are DMA'd later (after the first weight stream) — they
            # aren't needed until the first strip, ~25us in
            ident_t = consts.tile([128, 128], BF16, tag="ident")
            maskn_t = consts.tile([128, 128], BF16, tag="maskn")
            ones_t = consts.tile([128, 128], BF16, tag="ones")
            consts_loaded = [False]

            for _rep in range(reps):
                qT = [qkv.tile([128, S], BF16, tag=f"qT{p}", name=f"qT{p}") for p in range(HP)]
                kT = [qkv.tile([128, S], BF16, tag=f"kT{p}", name=f"kT{p}") for p in range(HP)]
                # v augmented with ones column: [128, head, jt, 65]
                # per-chunk tiles: dep tracking is tile-granular, so later
                # proj writes must not alias tiles earlier stages read
                v_aug = [qkv.tile([128, HC, CH // 128, D + 1], BF16,
                                  tag=f"v_aug{sc}", name=f"v_aug{sc}")
                         for sc in range(IT)]
                xT_out = [[xtop.tile([128, CH], BF16, tag=f"xto{p}_{sc}",
                                     name=f"xto{p}_{sc}")
                           for sc in range(IT)] for p in range(HP)]

                # ---- weights: preload everything (bf16: 32KB/partition) ----
                def load_w(wdram, ets=range(8), tiles=None):
                    tiles = list(tiles) if tiles is not None else [None] * 8
                    for et in ets:
                        t = wpool.tile([128, GD], BF16, tag="w")
                        eng = nc.sync if et % 2 == 0 else nc.scalar
                        eng.dma_start(out=t, in_=wdram[et * 128:(et + 1) * 128, :])
                        tiles[et] = t
                    return tiles

                def transpose_chunk(xdram, sc, eng2=None):
                    """DMA xT columns [sc*CH, (sc+1)*CH) into a slab [128, 8, CH].

                    eng2 defaults to sync: prefetches issued mid-strip must not
                    ride the ACT queue (it carries the exp stream)."""
                    slab = slabp.tile([128, 8, CH], BF16, tag="slab")
                    src = xdram.rearrange("(a p) s -> p a s", p=128)
                    (eng2 or nc.sync).dma_start(
                        out=slab[:, 0:4], in_=src[:, 0:4, sc * CH:(sc + 1) * CH])
                    nc.sync.dma_start(
                        out=slab[:, 4:8], in_=src[:, 4:8, sc * CH:(sc + 1) * CH])
                    return slab

                def proj_qk(dest, wt, slab, sc):
                    for dp in range(HP):
                        pj = ps.tile([128, 2, 512], F32, tag="ps", name="pj")
                        for et in range(8):
                            nc.tensor.matmul(
                                pj[:, 0, :],
                                wt[et][:, dp * 128:(dp + 1) * 128],
                                slab[:, et, :],
                                start=(et == 0), stop=(et == 7))
                        nc.vector.tensor_copy(
                            dest[dp][:, sc * CH:(sc + 1) * CH], pj[:, 0, :])

                def proj_v(wt, slab, sc):
                    for st in range(CH // 128):
                        pj = ps.tile([128, 2, 512], F32, tag="ps", name="pj")
                        for et in range(8):
                            nc.tensor.matmul(
                                pj[:, 0, :],
                                slab[:, et, st * 128:(st + 1) * 128],
                                wt[et],
                                start=(et == 0), stop=(et == 7))
                        nc.vector.tensor_copy(
                            v_aug[sc][:, :, st, 0:D],
                            pj[:, 0, :].rearrange("p (h d) -> p h d", h=HC))

                # ---- attention strip helpers ----
                def emit_scores(p, it, jt):
                    kdiag = jt - 4 * it
                    c0 = 128 * kdiag if kdiag > 0 else 0
                    i0 = it * 512 + c0
                    diag = kdiag >= 0
                    sw = ps.tile([128, 2, 512], F32, tag="ps", name="sw")
                    for hh in range(2):
                        nc.tensor.matmul(
                            sw[:, hh, c0:],
                            kT[p][hh * 64:(hh + 1) * 64, jt * 128:(jt + 1) * 128],
                            qT[p][hh * 64:(hh + 1) * 64, i0:(it + 1) * 512],
                            start=True, stop=not diag,
                            skip_group_check=diag)
                    if diag:
                        # additive causal mask on the diagonal block: the
                        # matmul adds maskneg[j, i] (-1e5 where j > i) into the
                        # first 128 score columns; exp then zeroes them
                        for hh in range(2):
                            nc.tensor.matmul(
                                sw[:, hh, c0:c0 + 128], ident_t, maskn_t,
                                start=False, stop=True, skip_group_check=True)
                    return sw, c0

                def emit_normalize(p, it, px1, px2, act_recip=False):
                    # act_recip: at chunk boundaries ACT is idle (no exp) and
                    # outproj waits on this chain — use ACT for the recips so
                    # both heads' chains overlap with the DVE muls.
                    rrows = []
                    for hh, px in ((0, px1), (1, px2)):
                        rrow = small.tile([1, 512], F32, tag="rrow", name="rrow")
                        nc.vector.reciprocal(rrow, px[64:65, :])
                        rrows.append(rrow)
                    for hh, px in ((0, px1), (1, px2)):
                        bc = small.tile([64, 512], F32, tag="bc", name="bc")
                        nc.gpsimd.partition_broadcast(bc, rrows[hh])
                        nc.vector.tensor_mul(
                            xT_out[p][it][hh * 64:(hh + 1) * 64, :],
                            px[0:64, :], bc)

                def emit_pv(p, it, jt, px1, px2, pt, c0):
                    h1, h2 = 2 * p, 2 * p + 1
                    jmax = 4 * it + 3
                    nc.tensor.matmul(
                        px1[0:65, c0:], v_aug[jt // 4][:, h1, jt % 4, :],
                        pt[:, 0, c0:],
                        start=(jt == 0), stop=(jt == jmax))
                    nc.tensor.matmul(
                        px2[0:65, c0:], v_aug[jt // 4][:, h2, jt % 4, :],
                        pt[:, 1, c0:],
                        start=(jt == 0), stop=(jt == jmax))

                def emit_outproj_tile(st, wot, split_store=False):
                    po = ps.tile([128, 2, 512], F32, tag="ps", name="po")
                    blk, col = st // 4, (st % 4) * 128
                    for eh in range(2):
                        for kt in range(4):
                            nc.tensor.matmul(
                                po[:, eh, :],
                                xT_out[kt][blk][:, col:col + 128],
                                wot[kt * 2 + eh],
                                start=(kt == 0), stop=(kt == 3))
                    # GPSIMD has no PSUM port — staging stays on DVE
                    ot = ostage.tile([128, 1024], BF16, tag="ostage")
                    if split_store:
                        nc.vector.tensor_copy(ot[:, 0:512], po[:, 0, :])
                        nc.sync.dma_start(
                            out=out[st * 128:(st + 1) * 128, 0:512],
                            in_=ot[:, 0:512])
                        nc.vector.tensor_copy(ot[:, 512:1024], po[:, 1, :])
                        nc.scalar.dma_start(
                            out=out[st * 128:(st + 1) * 128, 512:1024],
                            in_=ot[:, 512:1024])
                    else:
                        # sync only: these run during strips, where a DMA
                        # dispatch on ACT would stall the exp stream
                        nc.vector.tensor_copy(ot, po.rearrange("p a b -> p (a b)"))
                        nc.sync.dma_start(out=out[st * 128:(st + 1) * 128, :], in_=ot)

                def emit_qproj_group(sc, dp, slab):
                    pj = ps.tile([128, 2, 512], F32, tag="ps", name="pj")
                    for et in range(8):
                        nc.tensor.matmul(
                            pj[:, 0, :],
                            wts["q"][et][:, dp * 128:(dp + 1) * 128],
                            slab[:, et, :],
                            start=(et == 0), stop=(et == 7))
                    nc.vector.tensor_copy(
                        qT[dp][:, sc * CH:(sc + 1) * CH], pj[:, 0, :])

                # ---- fused schedule ----
                with nc.named_scope("mha"):
                    # startup: first two wq tiles, then the first slab, then
                    # the rest — the et=0 matmul can start as soon as its
                    # slab half + wq[0] land
                    wts = {"q": load_w(wq, ets=(0, 1))}
                    # chunk-0 q slab in quarters across both queues so the
                    # first projection matmul isn't starved
                    slab_q0 = slabp.tile([128, 8, CH], BF16, tag="slab")
                    srcq = xq.rearrange("(a p) s -> p a s", p=128)
                    for qi, eng in ((0, nc.sync), (1, nc.scalar),
                                    (2, nc.sync), (3, nc.scalar)):
                        eng.dma_start(out=slab_q0[:, 2 * qi:2 * qi + 2],
                                      in_=srcq[:, 2 * qi:2 * qi + 2, 0:CH])
                    wts["q"] = load_w(wq, ets=range(2, 8), tiles=wts["q"])
                    slab_k0 = transpose_chunk(xk, 0, eng2=nc.scalar)
                    wts["k"] = load_w(wk)
                    slab_v0 = transpose_chunk(xv, 0, eng2=nc.scalar)
                    wts["v"] = load_w(wv)
                    if not consts_loaded[0]:
                        nc.sync.dma_start(out=ident_t, in_=identd[:, :])
                        nc.scalar.dma_start(out=maskn_t, in_=masknd[:, :])
                        nc.scalar.dma_start(out=ones_t, in_=onesd[:, :])
                        consts_loaded[0] = True
                    for sc in range(IT):
                        nc.gpsimd.tensor_copy(
                            v_aug[sc][:, :, :, D:D + 1],
                            ones_t[:, 0:HC * (CH // 128)].rearrange(
                                "p (a b c) -> p a b c", a=HC, b=CH // 128))
                    wot = []
                    for kt in range(4):
                        for eh in range(2):
                            t = wpool.tile([128, GD], BF16, tag="w")
                            eng = nc.sync if eh == 0 else nc.scalar
                            eng.dma_start(
                                out=t, in_=wo[kt * 128:(kt + 1) * 128,
                                              eh * 512:(eh + 1) * 512])
                            wot.append(t)

                    # ---- one continuous attention pipeline ----
                    # all (pair, chunk, j-tile) stages form a single
                    # depth-2 score->exp->PV pipeline (ACT never drains at
                    # pair/chunk boundaries). k/v projection blocks are
                    # emitted as barriers right before the first scores of
                    # their chunk; pair transitions carry normalize +
                    # outproj-tile (chunk sc-1) + q-proj (chunk sc+1)
                    # fillers that keep PE busy while ACT catches up.
                    proj_qk(qT, wts["q"], slab_q0, 0)
                    proj_qk(kT, wts["k"], slab_k0, 0)
                    proj_v(wts["v"], slab_v0, 0)
                    slabs = {0: [slab_q0, slab_k0, slab_v0]}
                    slabs[1] = [transpose_chunk(xq, 1),
                                transpose_chunk(xk, 1),
                                transpose_chunk(xv, 1)]

                    stages = [(p, sc, jt)
                              for sc in range(IT)
                              for p in range(HP)
                              for jt in range(4 * sc + 4)]
                    first_of_chunk = {}
                    for idx, (p, sc, jt) in enumerate(stages):
                        first_of_chunk.setdefault(sc, idx)

                    pend = []      # [(sw, c0), ...] scores not yet consumed
                    px_cur = {}    # pair -> (px1, px2)

                    def emit_stage_scores(idx):
                        p, sc, jt = stages[idx]
                        if sc >= 1 and idx == first_of_chunk[sc]:
                            # barrier: this chunk's k/v projections (q was
                            # filled at chunk sc-1 pair transitions)
                            proj_qk(kT, wts["k"], slabs[sc][1], sc)
                            proj_v(wts["v"], slabs[sc][2], sc)
                            if sc + 1 < IT:
                                slabs[sc + 1] = [transpose_chunk(xq, sc + 1),
                                                 transpose_chunk(xk, sc + 1),
                                                 transpose_chunk(xv, sc + 1)]
                        pend.append(emit_scores(p, sc, jt))

                    emit_stage_scores(0)
                    emit_stage_scores(1)
                    for i, (p, sc, jt) in enumerate(stages):
                        sw_cur, c0 = pend.pop(0)
                        pt = ptp.tile([128, 2, 512], BF16, tag="pt", name="pt")
                        nc.scalar.activation(pt[:, :, c0:], sw_cur[:, :, c0:],
                                             EXP, scale=SCALE)
                        if i + 2 < len(stages):
                            emit_stage_scores(i + 2)
                        if jt == 0:
                            px_cur[p] = (
                                psx.tile([128, 512], F32, tag="psx", name="px1"),
                                psx.tile([128, 512], F32, tag="psx", name="px2"))
                        px1, px2 = px_cur[p]
                        emit_pv(p, sc, jt, px1, px2, pt, c0)
                        if jt == 4 * sc + 3:
                            # pair transition: normalize, then filler blocks.
                            # outproj tiles all go to chunk-3 transitions
                            # (3 each) — that's where ACT-pacing leaves PE
                            # idle and no q-proj filler remains.
                            emit_normalize(p, sc, px1, px2)
                            if sc + 1 < IT:
                                emit_qproj_group(sc + 1, p, slabs[sc + 1][0])
                            if sc == IT - 1:
                                for st in range(3 * p, 3 * p + 3):
                                    emit_outproj_tile(st, wot)
                    for st in range(4 * (IT - 1), 4 * IT):
                        emit_outproj_tile(st, wot, split_store=True)

    nc.finalize()
    return nc


_NC = None


def _get_nc():
    global _NC
    if _NC is None:
        _NC = build_core_kernel()
    return _NC


def _maskneg():
    # additive causal mask for s^T blocks: -1e5 where j (row) > i (col)
    r = np.arange(128)
    return np.where(r[:, None] > r[None, :], -1.0e5, 0.0).astype(ml_dtypes.bfloat16)


def make_in_maps(query, key, value, Wq, Wk, Wv, Wo):
    bf = ml_dtypes.bfloat16
    query = np.asarray(query, np.float32)
    key = np.asarray(key, np.float32)
    value = np.asarray(value, np.float32)
    Wq = np.asarray(Wq, np.float32).astype(bf)
    Wk = np.asarray(Wk, np.float32).astype(bf)
    Wv = np.asarray(Wv, np.float32).astype(bf)
    Wo = np.asarray(Wo, np.float32).astype(bf)
    maskn_m = _maskneg()
    ident_m = np.eye(128, dtype=np.float32).astype(bf)
    ones_m = np.ones((128, 128), bf)
    xqT = [np.ascontiguousarray(query[b].T.astype(bf)) for b in range(B)]
    xkT = [np.ascontiguousarray(key[b].T.astype(bf)) for b in range(B)]
    xvT = [np.ascontiguousarray(value[b].T.astype(bf)) for b in range(B)]
    in_maps = []
    for c in range(N_CORES):
        b, g = c // 2, c % 2
        cols = slice(g * GD, (g + 1) * GD)
        in_maps.append({
            "xqT": xqT[b],
            "xkT": xkT[b],
            "xvT": xvT[b],
            "wq": np.ascontiguousarray(Wq[:, cols]),
            "wk": np.ascontiguousarray(Wk[:, cols]),
            "wv": np.ascontiguousarray(Wv[:, cols]),
            "wo": np.ascontiguousarray(Wo[g * GD:(g + 1) * GD, :]),
            "identd": ident_m,
            "maskneg": maskn_m,
            "onesc": ones_m,
        })
    return in_maps


def kernel(query, key, value, mask, Wq, Wk, Wv, Wo, **run_kwargs):
    nc = _get_nc()
    in_maps = make_in_maps(query, key, value, Wq, Wk, Wv, Wo)
    res = run_bass_kernel_spmd(nc, in_maps, core_ids=list(range(N_CORES)),
                               **run_kwargs)
    out = np.empty((B, S, E), np.float32)
    for b in range(B):
        out[b] = (res.results[2 * b]["out"].astype(np.float32)
                  + res.results[2 * b + 1]["out"].astype(np.float32))
    if run_kwargs:
        kernel.last_result = res
    return out


if __name__ == "__main__":
    rng = np.random.default_rng(0)
    q = rng.standard_normal((B, S, E), dtype=np.float32)
    k = rng.standard_normal((B, S, E), dtype=np.float32)
    v = rng.standard_normal((B, S, E), dtype=np.float32)
    sc = 1.0 / np.sqrt(E)
    Wq = rng.standard_normal((E, E), dtype=np.float32) * sc
    Wk = rng.standard_normal((E, E), dtype=np.float32) * sc
    Wv = rng.standard_normal((E, E), dtype=np.float32) * sc
    Wo = rng.standard_normal((E, E), dtype=np.float32) * sc
    o = kernel(q, k, v, None, Wq, Wk, Wv, Wo)
    print("out", o.shape, o.dtype, float(np.abs(o).mean()))
